# Initial kernel scaffold
#
"""TRN2 Bass kernel for nn_Decoder_SICA (dense CNN decoder), 8-core data parallel.

Network (per sample):
  stage0: per-sample grouped conv_transpose (stride==kernel) == block einsum
          A(512,2,2) x S(64,3,5,5) -> x0 (384,10,10)
  conv1:  384->384 3x3 pad1        -> (384,10,10) relu
  conv2:  384->384 3x3 s2 pad1     -> (384,5,5)   relu
  conv3:  384->512 3x3 pad1        -> (512,5,5)   relu
  conv4:  512->512 5x5 valid       -> (512,1,1)   relu
  linear: 512->10

Design notes:
  - batch dim sharded 8 ways (64 samples/core), weights replicated.
  - activations live in SBUF as [C_part=128, (c_tile), H, W, B] - batch
    INNERMOST, which satisfies the fp32r ISA rules (moving innermost count
    even, psum dst innermost stride 1 / even count, outer steps even).
  - convs run as accumulation groups of per-offset matmuls with
    boundary-split output rectangles (no padding, no im2col copies); PSUM's
    per-element has_written bit makes partial-coverage accumulation exact.
  - dtype float32r (fp32 data, ~1e-4 matmul rel err, bf16-speed for N>=256).
    stage0 uses bf16 (tiny einsum; its stationary operand changes per
    sample so LDWEIGHTS dominates; bf16 halves that).
  - stage0's per-sample contraction (g,c)=64 is made a single K=64 matmul
    per (h,w) via a host-built block-diagonal stationary matrix from A
    (pure data movement on host, zero on-chip shuffling).
"""

import numpy as np
import ml_dtypes

import concourse.bacc as bacc
import concourse.mybir as mybir
from concourse.tile import TileContext
from concourse.bass_utils import run_bass_kernel_spmd

P = 128
B_FULL = 512
NCORES = 8
BL = B_FULL // NCORES        # 64 samples per core
SLAB = 32                    # stage0+conv1 batch slab
F32R = mybir.dt.float32r
BF16 = mybir.dt.bfloat16
FP32 = mybir.dt.float32


def _build_program(loop_n=1, accum_out=False):
    nc = bacc.Bacc("TRN2", target_bir_lowering=False, debug=False,
                   num_devices=NCORES)

    # ---- DRAM I/O (per core) ----
    abd = nc.dram_tensor("abd", [BL // 4, 64, 4, 4 * P], BF16, kind="ExternalInput")
    s2 = nc.dram_tensor("s2", [BL // 4, 64, 4, 75], BF16, kind="ExternalInput")
    w1t = nc.dram_tensor("w1t", [3, P, 3, 3, 3, P], F32R, kind="ExternalInput")
    w2t = nc.dram_tensor("w2t", [3, P, 3, 3, 3, P], F32R, kind="ExternalInput")
    w3t = nc.dram_tensor("w3t", [4, P, 3, 3, 3, P], F32R, kind="ExternalInput")
    w4t = nc.dram_tensor("w4t", [4, P, 4, 5, 5, P], BF16, kind="ExternalInput")
    wl = nc.dram_tensor("wl", [P, 4, 10], F32R, kind="ExternalInput")
    biases = nc.dram_tensor("biases", [P, 14], FP32, kind="ExternalInput")
    blrep = nc.dram_tensor("blrep", [BL, 10], FP32, kind="ExternalInput")
    out_d = nc.dram_tensor("OUT", [BL, 10], FP32, kind="ExternalOutput")
    # bias columns: b1 -> 0:3, b2 -> 3:6, b3 -> 6:10, b4 -> 10:14

    RELU = mybir.ActivationFunctionType.Relu

    with TileContext(nc) as tc:
        with (
            tc.tile_pool(name="acts_a", bufs=1) as pool_a,   # x0 slabs, then x2
            tc.tile_pool(name="acts_b", bufs=1) as pool_b,   # x1, then x3
            tc.tile_pool(name="wpool", bufs=3) as wpool,     # weight streaming
            tc.tile_pool(name="s0pool", bufs=6) as s0pool,   # stage0 A/S staging
            tc.tile_pool(name="misc", bufs=1) as misc,
            tc.tile_pool(name="pspool", bufs=3, space="PSUM") as pspool,
            tc.tile_pool(name="ps0pool", bufs=4, space="PSUM") as ps0pool,
            tc.tile_pool(name="pslin", bufs=1, space="PSUM") as pslin,
        ):
            bias_t = misc.tile([P, 14], FP32, name="bias_t")
            nc.sync.dma_start(bias_t[:], biases[:])
            wl_t = misc.tile([P, 4, 10], F32R, name="wl_t")
            nc.sync.dma_start(wl_t[:], wl[:])
            bl_t = misc.tile([BL, 10], FP32, name="bl_t")
            nc.sync.dma_start(bl_t[:], blrep[:])

            def body():
                _emit_body(nc, tc, pool_a, pool_b, wpool, s0pool, misc,
                           pspool, ps0pool, pslin, bias_t, wl_t, bl_t,
                           abd, s2, w1t, w2t, w3t, w4t, out_d, RELU,
                           accum_out)

            if loop_n > 1:
                with tc.For_i(0, loop_n, 1):
                    body()
            else:
                body()

    nc.compile()
    return nc


def _emit_body(nc, tc, pool_a, pool_b, wpool, s0pool, misc, pspool, ps0pool,
               pslin, bias_t, wl_t, bl_t, abd, s2, w1t, w2t, w3t, w4t, out_d,
               RELU, accum_out=False):
    if True:
        if True:
            x1 = pool_b.tile([P, 3, 10, 10, BL], F32R, name="x1", tag="bufB")

            # ---- per-slab: stage0 + conv1 ----
            for slab in range(BL // SLAB):
                b0 = slab * SLAB
                x0s = pool_a.tile([P, 3, 10, 10, SLAB], F32R, name=f"x0_{slab}",
                                  tag="bufA")
                # stage0: per sample, 4 matmuls (one per (h,w)) into one
                # psum laid out (o, row, col); one copy evicts to x0s[..., b].
                SG = 4  # samples per staging tile
                for bg in range(SLAB // SG):
                    b = b0 + bg * SG
                    st = s0pool.tile([64, SG, 4 * P + 75], BF16,
                                     name=f"st{b}", tag="st")
                    g = b // SG
                    nc.sync.dma_start(st[:, :, : 4 * P], abd[g])
                    nc.sync.dma_start(st[:, :, 4 * P :], s2[g])
                    for bi in range(SG):
                        ps0 = ps0pool.tile([P, 3, 10, 10], FP32,
                                           name=f"ps0_{b + bi}", tag="ps0")
                        for hw in range(4):
                            h, w = hw // 2, hw % 2
                            nc.tensor.matmul(
                                ps0[:, :, 5 * h : 5 * h + 5,
                                    5 * w : 5 * w + 5],
                                lhsT=st[:, bi, hw * P : (hw + 1) * P],
                                rhs=st[:, bi, 4 * P :],
                                start=(hw == 0),
                                stop=(hw == 3),
                            )
                        nc.scalar.copy(
                            x0s[:, :, :, :, bg * SG + bi], ps0[:])

                # conv1: 384->384 3x3 pad1 on 10x10, batch slab of 32.
                # psum = one output row [co, 10, 32]; offsets boundary-split.
                for cot in range(3):
                    w_t = wpool.tile([P, 3, 3, 3, P], F32R,
                                     name=f"w1_{slab}_{cot}", tag="w")
                    nc.sync.dma_start(w_t[:], w1t[cot])
                    for r in range(10):
                        pt = pspool.tile([P, 10, SLAB], FP32,
                                         name=f"p1_{slab}_{cot}_{r}", tag="ps")
                        first = True
                        for cit in range(3):
                            for dh in (-1, 0, 1):
                                ir = r + dh
                                if ir < 0 or ir > 9:
                                    continue
                                for dw in (-1, 0, 1):
                                    ow0, own = max(0, -dw), min(10, 10 - dw)
                                    iw0 = ow0 + dw
                                    nw = own - ow0
                                    nc.tensor.matmul(
                                        pt[:, ow0:own, :],
                                        lhsT=w_t[:, cit, dh + 1, dw + 1, :],
                                        rhs=x0s[:, cit, ir, iw0 : iw0 + nw, :],
                                        start=first,
                                        stop=False,
                                        skip_group_check=True,
                                    )
                                    first = False
                        nc.scalar.activation(
                            x1[:, cot, r, :, b0 : b0 + SLAB], pt[:], RELU,
                            bias=bias_t[:, cot : cot + 1],
                        )

            # ---- conv2: 384->384 3x3 stride2 pad1, 10x10 -> 5x5 ----
            x2 = pool_a.tile([P, 3, 5, 5, BL], F32R, name="x2", tag="bufA")
            for cot in range(3):
                w_t = wpool.tile([P, 3, 3, 3, P], F32R, name=f"w2_{cot}",
                                 tag="w")
                nc.sync.dma_start(w_t[:], w2t[cot])
                for r in range(5):
                    pt = pspool.tile([P, 5, BL], FP32, name=f"p2_{cot}_{r}",
                                     tag="ps")
                    first = True
                    for cit in range(3):
                        for dh in (-1, 0, 1):
                            ir = 2 * r + dh
                            if ir < 0 or ir > 9:
                                continue
                            for dw in (-1, 0, 1):
                                ow0 = 1 if dw == -1 else 0
                                nw = 5 - ow0
                                iw0 = 2 * ow0 + dw
                                nc.tensor.matmul(
                                    pt[:, ow0:5, :],
                                    lhsT=w_t[:, cit, dh + 1, dw + 1, :],
                                    rhs=x1[:, cit, ir,
                                           iw0 : iw0 + 2 * nw - 1 : 2, :],
                                    start=first,
                                    stop=False,
                                    skip_group_check=True,
                                )
                                first = False
                    nc.scalar.activation(
                        x2[:, cot, r, :, :], pt[:], RELU,
                        bias=bias_t[:, 3 + cot : 4 + cot],
                    )

            # ---- conv3: 384->512 3x3 pad1 on 5x5 ----
            x3 = pool_b.tile([P, 4, 5, 5, BL], BF16, name="x3", tag="bufB")
            for cot in range(4):
                w_t = wpool.tile([P, 3, 3, 3, P], F32R, name=f"w3_{cot}",
                                 tag="w")
                nc.sync.dma_start(w_t[:], w3t[cot])
                for r in range(5):
                    pt = pspool.tile([P, 5, BL], FP32, name=f"p3_{cot}_{r}",
                                     tag="ps")
                    first = True
                    for cit in range(3):
                        for dh in (-1, 0, 1):
                            ir = r + dh
                            if ir < 0 or ir > 4:
                                continue
                            for dw in (-1, 0, 1):
                                ow0, own = max(0, -dw), min(5, 5 - dw)
                                iw0 = ow0 + dw
                                nw = own - ow0
                                nc.tensor.matmul(
                                    pt[:, ow0:own, :],
                                    lhsT=w_t[:, cit, dh + 1, dw + 1, :],
                                    rhs=x2[:, cit, ir, iw0 : iw0 + nw, :],
                                    start=first,
                                    stop=False,
                                    skip_group_check=True,
                                )
                                first = False
                    nc.scalar.activation(
                        x3[:, cot, r, :, :], pt[:], RELU,
                        bias=bias_t[:, 6 + cot : 7 + cot],
                    )

            # ---- conv4: 512->512 5x5 valid, 5x5 -> 1x1 ----
            x4 = misc.tile([P, 4, BL], F32R, name="x4")
            for cot in range(4):
                pt = pspool.tile([P, BL], FP32, name=f"p4_{cot}", tag="ps")
                first = True
                for cit in range(4):
                    w_t = wpool.tile([P, 5, 5, P], BF16,
                                     name=f"w4_{cot}_{cit}", tag="w")
                    nc.sync.dma_start(w_t[:], w4t[cot, :, cit])
                    for rr in range(5):
                        for cc in range(5):
                            nc.tensor.matmul(
                                pt[:],
                                lhsT=w_t[:, rr, cc, :],
                                rhs=x3[:, cit, rr, cc, :],
                                start=first,
                                stop=(cit == 3 and rr == 4 and cc == 4),
                            )
                            first = False
                nc.scalar.activation(
                    x4[:, cot, :], pt[:], RELU,
                    bias=bias_t[:, 10 + cot : 11 + cot],
                )

            # ---- linear: 512 -> 10 ----
            pl = pslin.tile([BL, 10], FP32, name="pl")
            for cit in range(4):
                nc.tensor.matmul(
                    pl[:],
                    lhsT=x4[:, cit, :],
                    rhs=wl_t[:, cit, :],
                    start=(cit == 0),
                    stop=(cit == 3),
                )
            out_sb = misc.tile([BL, 10], FP32, name="out_sb")
            nc.vector.tensor_add(out_sb[:], pl[:], bl_t[:])
            if accum_out:
                # accumulate into DRAM so R loop iterations yield R*result -
                # verifies the in-NEFF timing loop actually executed R times
                nc.gpsimd.dma_start(out_d[:], out_sb[:],
                                    accum_op=mybir.AluOpType.add)
            else:
                nc.sync.dma_start(out_d[:], out_sb[:])


def _prep_core_inputs(A, S, W1, b1, W2, b2, W3, b3, W4, b4, Wl, bl):
    """Host-side reshapes (pure data movement). Returns list of per-core dicts."""
    B = A.shape[0]
    # stage0 block-diagonal stationary from A:
    # abd[b, hw, g*4+c, n*16+g] = A[b, n*64+g*4+c, h, w]
    A_r = np.ascontiguousarray(
        A.reshape(B, 8, 16, 4, 4).transpose(0, 4, 2, 3, 1)
    )  # [B, hw, g, c, n]
    abd = np.zeros((B, 4, 64, P), dtype=ml_dtypes.bfloat16)
    abd_v = abd.reshape(B, 4, 16, 4, 8, 16)  # [B, hw, g, c, n, g']
    gg = np.arange(16)
    abd_v[:, :, gg, :, :, gg] = A_r.transpose(2, 0, 1, 3, 4)[gg].astype(
        ml_dtypes.bfloat16
    )
    # group-major staging layout: [B/4, part, b_in_group, hw*128]
    abd = np.ascontiguousarray(
        abd.reshape(B // 4, 4, 4, 64, P).transpose(0, 3, 1, 2, 4)
    ).reshape(B // 4, 64, 4, 4 * P)
    s2 = S.reshape(B, 64, 75).astype(ml_dtypes.bfloat16)
    s2 = np.ascontiguousarray(
        s2.reshape(B // 4, 4, 64, 75).transpose(0, 2, 1, 3)
    )

    def conv_w_tiles(W, n_cot, n_cit, k):
        co, ci = W.shape[0], W.shape[1]
        # -> [cot, cip, cit, kh, kw, cof]
        t = W.reshape(n_cot, P, n_cit, P, k, k).transpose(0, 3, 2, 4, 5, 1)
        return np.ascontiguousarray(t).astype(np.float32)

    # x0's channel layout from stage0 is (o, n*16+g); permute W1's ci to match
    # (reference ci index = n*48 + g*3 + o).
    o_i, n_i, g_i = np.meshgrid(
        np.arange(3), np.arange(8), np.arange(16), indexing="ij"
    )
    perm = (n_i * 48 + g_i * 3 + o_i).reshape(-1)
    w1t = conv_w_tiles(W1[:, perm], 3, 3, 3)
    w2t = conv_w_tiles(W2, 3, 3, 3)
    w3t = conv_w_tiles(W3, 4, 3, 3)
    w4t = conv_w_tiles(W4, 4, 4, 5).astype(ml_dtypes.bfloat16)

    wl_a = np.ascontiguousarray(
        Wl.T.reshape(4, P, 10).transpose(1, 0, 2)
    ).astype(np.float32)
    biases = np.zeros((P, 14), np.float32)
    biases[:, 0:3] = b1.reshape(3, P).T
    biases[:, 3:6] = b2.reshape(3, P).T
    biases[:, 6:10] = b3.reshape(4, P).T
    biases[:, 10:14] = b4.reshape(4, P).T
    blrep = np.tile(bl.astype(np.float32), (BL, 1))

    in_maps = []
    for c in range(NCORES):
        sl = slice(c * BL // 4, (c + 1) * BL // 4)
        in_maps.append({
            "abd": abd[sl], "s2": s2[sl],
            "w1t": w1t, "w2t": w2t, "w3t": w3t, "w4t": w4t,
            "wl": wl_a, "biases": biases, "blrep": blrep,
        })
    return in_maps


_PROGRAM_CACHE = {}


def _get_program():
    if "nc" not in _PROGRAM_CACHE:
        _PROGRAM_CACHE["nc"] = _build_program()
    return _PROGRAM_CACHE["nc"]


def kernel(A, S, W1, b1, W2, b2, W3, b3, W4, b4, Wl, bl):
    A = np.asarray(A, np.float32)
    S = np.asarray(S, np.float32)
    in_maps = _prep_core_inputs(
        A, S,
        np.asarray(W1, np.float32), np.asarray(b1, np.float32),
        np.asarray(W2, np.float32), np.asarray(b2, np.float32),
        np.asarray(W3, np.float32), np.asarray(b3, np.float32),
        np.asarray(W4, np.float32), np.asarray(b4, np.float32),
        np.asarray(Wl, np.float32), np.asarray(bl, np.float32),
    )
    nc = _get_program()
    res = run_bass_kernel_spmd(nc, in_maps, list(range(NCORES)))
    return np.concatenate([res.results[c]["OUT"] for c in range(NCORES)], axis=0)



# revision 1
# speedup vs baseline: 1.6494x; 1.6494x over previous
"""TRN2 Bass kernel for nn_Decoder_SICA (dense CNN decoder), 8-core data parallel.

Network (per sample):
  stage0: per-sample grouped conv_transpose (stride==kernel) == block einsum
          A(512,2,2) x S(64,3,5,5) -> x0 (384,10,10)
  conv1:  384->384 3x3 pad1        -> (384,10,10) relu
  conv2:  384->384 3x3 s2 pad1     -> (384,5,5)   relu
  conv3:  384->512 3x3 pad1        -> (512,5,5)   relu
  conv4:  512->512 5x5 valid       -> (512,1,1)   relu
  linear: 512->10

Design notes:
  - batch dim sharded 8 ways (64 samples/core), weights replicated.
  - activations live in SBUF as [C_part=128, (c_tile), H, W, B] - batch
    INNERMOST, which satisfies the fp32r ISA rules (moving innermost count
    even, psum dst innermost stride 1 / even count, outer steps even).
  - convs run as accumulation groups of per-offset matmuls with
    boundary-split output rectangles (no padding, no im2col copies); PSUM's
    per-element has_written bit makes partial-coverage accumulation exact.
  - dtype float32r (fp32 data, ~1e-4 matmul rel err, bf16-speed for N>=256).
    stage0 uses bf16 (tiny einsum; its stationary operand changes per
    sample so LDWEIGHTS dominates; bf16 halves that).
  - stage0's per-sample contraction (g,c)=64 is made a single K=64 matmul
    per (h,w) via a host-built block-diagonal stationary matrix from A
    (pure data movement on host, zero on-chip shuffling).
"""

import numpy as np
import ml_dtypes

import concourse.bacc as bacc
import concourse.mybir as mybir
from concourse.tile import TileContext
from concourse.bass_utils import run_bass_kernel_spmd

P = 128
B_FULL = 512
NCORES = 8
BL = B_FULL // NCORES        # 64 samples per core
SLAB = 32                    # stage0+conv1 batch slab
F32R = mybir.dt.float32r
BF16 = mybir.dt.bfloat16
FP32 = mybir.dt.float32


def _build_program(loop_n=1, accum_out=False):
    nc = bacc.Bacc("TRN2", target_bir_lowering=False, debug=False,
                   num_devices=NCORES)

    # ---- DRAM I/O (per core) ----
    abd = nc.dram_tensor("abd", [BL // 4, 64, 4, 4 * P], BF16, kind="ExternalInput")
    s2 = nc.dram_tensor("s2", [BL // 4, 64, 4, 75], BF16, kind="ExternalInput")
    w1t = nc.dram_tensor("w1t", [3, P, 3, 3, 3, P], F32R, kind="ExternalInput")
    w2t = nc.dram_tensor("w2t", [3, P, 3, 3, 3, P], F32R, kind="ExternalInput")
    w3t = nc.dram_tensor("w3t", [4, P, 3, 3, 3, P], F32R, kind="ExternalInput")
    w4t = nc.dram_tensor("w4t", [4, P, 4, 5, 5, P], BF16, kind="ExternalInput")
    wl = nc.dram_tensor("wl", [P, 4, 10], F32R, kind="ExternalInput")
    biases = nc.dram_tensor("biases", [P, 14], FP32, kind="ExternalInput")
    blrep = nc.dram_tensor("blrep", [BL, 10], FP32, kind="ExternalInput")
    out_d = nc.dram_tensor("OUT", [BL, 10], FP32, kind="ExternalOutput")
    # bias columns: b1 -> 0:3, b2 -> 3:6, b3 -> 6:10, b4 -> 10:14

    RELU = mybir.ActivationFunctionType.Relu

    with TileContext(nc) as tc:
        with (
            tc.tile_pool(name="acts_a", bufs=1) as pool_a,   # x0 slabs, then x2
            tc.tile_pool(name="acts_b", bufs=1) as pool_b,   # x1, then x3
            tc.tile_pool(name="wpool", bufs=3) as wpool,     # weight streaming
            tc.tile_pool(name="s0pool", bufs=6) as s0pool,   # stage0 A/S staging
            tc.tile_pool(name="misc", bufs=1) as misc,
            tc.tile_pool(name="pspool", bufs=3, space="PSUM") as pspool,
            tc.tile_pool(name="ps0pool", bufs=4, space="PSUM") as ps0pool,
            tc.tile_pool(name="pslin", bufs=1, space="PSUM") as pslin,
        ):
            bias_t = misc.tile([P, 14], FP32, name="bias_t")
            nc.sync.dma_start(bias_t[:], biases[:])
            wl_t = misc.tile([P, 4, 10], F32R, name="wl_t")
            nc.sync.dma_start(wl_t[:], wl[:])
            bl_t = misc.tile([BL, 10], FP32, name="bl_t")
            nc.sync.dma_start(bl_t[:], blrep[:])

            def body():
                _emit_body(nc, tc, pool_a, pool_b, wpool, s0pool, misc,
                           pspool, ps0pool, pslin, bias_t, wl_t, bl_t,
                           abd, s2, w1t, w2t, w3t, w4t, out_d, RELU,
                           accum_out)

            if loop_n > 1:
                with tc.For_i(0, loop_n, 1):
                    body()
            else:
                body()

    nc.compile()
    return nc


def _emit_body(nc, tc, pool_a, pool_b, wpool, s0pool, misc, pspool, ps0pool,
               pslin, bias_t, wl_t, bl_t, abd, s2, w1t, w2t, w3t, w4t, out_d,
               RELU, accum_out=False):
    if True:
        if True:
            x1 = pool_b.tile([P, 3, 10, 10, BL], F32R, name="x1", tag="bufB")

            # ---- per-slab: stage0 + conv1 ----
            for slab in range(BL // SLAB):
                b0 = slab * SLAB
                x0s = pool_a.tile([P, 3, 10, 10, SLAB], F32R, name=f"x0_{slab}",
                                  tag="bufA")
                # stage0: per sample, 4 matmuls (one per (h,w)) into one
                # psum laid out (o, row, col); one copy evicts to x0s[..., b].
                SG = 4  # samples per staging tile
                for bg in range(SLAB // SG):
                    b = b0 + bg * SG
                    st = s0pool.tile([64, SG, 4 * P + 75], BF16,
                                     name=f"st{b}", tag="st")
                    g = b // SG
                    nc.sync.dma_start(st[:, :, : 4 * P], abd[g])
                    nc.sync.dma_start(st[:, :, 4 * P :], s2[g])
                    for bi in range(SG):
                        ps0 = ps0pool.tile([P, 3, 10, 10], FP32,
                                           name=f"ps0_{b + bi}", tag="ps0")
                        for hw in range(4):
                            h, w = hw // 2, hw % 2
                            nc.tensor.matmul(
                                ps0[:, :, 5 * h : 5 * h + 5,
                                    5 * w : 5 * w + 5],
                                lhsT=st[:, bi, hw * P : (hw + 1) * P],
                                rhs=st[:, bi, 4 * P :],
                                start=(hw == 0),
                                stop=(hw == 3),
                            )
                        nc.scalar.copy(
                            x0s[:, :, :, :, bg * SG + bi], ps0[:])

                # conv1: 384->384 3x3 pad1 on 10x10, batch slab of 32.
                # psum = one output row [co, 10, 32]; offsets boundary-split.
                for cot in range(3):
                    w_t = wpool.tile([P, 3, 3, 3, P], F32R,
                                     name=f"w1_{slab}_{cot}", tag="w")
                    nc.sync.dma_start(w_t[:], w1t[cot])
                    for r in range(10):
                        pt = pspool.tile([P, 10, SLAB], FP32,
                                         name=f"p1_{slab}_{cot}_{r}", tag="ps")
                        first = True
                        for cit in range(3):
                            for dh in (-1, 0, 1):
                                ir = r + dh
                                if ir < 0 or ir > 9:
                                    continue
                                for dw in (-1, 0, 1):
                                    ow0, own = max(0, -dw), min(10, 10 - dw)
                                    iw0 = ow0 + dw
                                    nw = own - ow0
                                    nc.tensor.matmul(
                                        pt[:, ow0:own, :],
                                        lhsT=w_t[:, cit, dh + 1, dw + 1, :],
                                        rhs=x0s[:, cit, ir, iw0 : iw0 + nw, :],
                                        start=first,
                                        stop=False,
                                        skip_group_check=True,
                                    )
                                    first = False
                        nc.scalar.activation(
                            x1[:, cot, r, :, b0 : b0 + SLAB], pt[:], RELU,
                            bias=bias_t[:, cot : cot + 1],
                        )

            # ---- conv2: 384->384 3x3 stride2 pad1, 10x10 -> 5x5 ----
            x2 = pool_a.tile([P, 3, 5, 5, BL], F32R, name="x2", tag="bufA")
            for cot in range(3):
                w_t = wpool.tile([P, 3, 3, 3, P], F32R, name=f"w2_{cot}",
                                 tag="w")
                nc.sync.dma_start(w_t[:], w2t[cot])
                for r in range(5):
                    pt = pspool.tile([P, 5, BL], FP32, name=f"p2_{cot}_{r}",
                                     tag="ps")
                    first = True
                    for cit in range(3):
                        for dh in (-1, 0, 1):
                            ir = 2 * r + dh
                            if ir < 0 or ir > 9:
                                continue
                            for dw in (-1, 0, 1):
                                ow0 = 1 if dw == -1 else 0
                                nw = 5 - ow0
                                iw0 = 2 * ow0 + dw
                                nc.tensor.matmul(
                                    pt[:, ow0:5, :],
                                    lhsT=w_t[:, cit, dh + 1, dw + 1, :],
                                    rhs=x1[:, cit, ir,
                                           iw0 : iw0 + 2 * nw - 1 : 2, :],
                                    start=first,
                                    stop=False,
                                    skip_group_check=True,
                                )
                                first = False
                    nc.scalar.activation(
                        x2[:, cot, r, :, :], pt[:], RELU,
                        bias=bias_t[:, 3 + cot : 4 + cot],
                    )

            # ---- conv3: 384->512 3x3 pad1 on 5x5 ----
            x3 = pool_b.tile([P, 4, 5, 5, BL], BF16, name="x3", tag="bufB")
            for cot in range(4):
                w_t = wpool.tile([P, 3, 3, 3, P], F32R, name=f"w3_{cot}",
                                 tag="w")
                nc.sync.dma_start(w_t[:], w3t[cot])
                for r in range(5):
                    pt = pspool.tile([P, 5, BL], FP32, name=f"p3_{cot}_{r}",
                                     tag="ps")
                    first = True
                    for cit in range(3):
                        for dh in (-1, 0, 1):
                            ir = r + dh
                            if ir < 0 or ir > 4:
                                continue
                            for dw in (-1, 0, 1):
                                ow0, own = max(0, -dw), min(5, 5 - dw)
                                iw0 = ow0 + dw
                                nw = own - ow0
                                nc.tensor.matmul(
                                    pt[:, ow0:own, :],
                                    lhsT=w_t[:, cit, dh + 1, dw + 1, :],
                                    rhs=x2[:, cit, ir, iw0 : iw0 + nw, :],
                                    start=first,
                                    stop=False,
                                    skip_group_check=True,
                                )
                                first = False
                    nc.scalar.activation(
                        x3[:, cot, r, :, :], pt[:], RELU,
                        bias=bias_t[:, 6 + cot : 7 + cot],
                    )

            # ---- conv4: 512->512 5x5 valid, 5x5 -> 1x1 ----
            x4 = misc.tile([P, 4, BL], F32R, name="x4")
            for cot in range(4):
                pt = pspool.tile([P, BL], FP32, name=f"p4_{cot}", tag="ps")
                first = True
                for cit in range(4):
                    w_t = wpool.tile([P, 5, 5, P], BF16,
                                     name=f"w4_{cot}_{cit}", tag="w")
                    nc.sync.dma_start(w_t[:], w4t[cot, :, cit])
                    for rr in range(5):
                        for cc in range(5):
                            nc.tensor.matmul(
                                pt[:],
                                lhsT=w_t[:, rr, cc, :],
                                rhs=x3[:, cit, rr, cc, :],
                                start=first,
                                stop=(cit == 3 and rr == 4 and cc == 4),
                            )
                            first = False
                nc.scalar.activation(
                    x4[:, cot, :], pt[:], RELU,
                    bias=bias_t[:, 10 + cot : 11 + cot],
                )

            # ---- linear: 512 -> 10 ----
            pl = pslin.tile([BL, 10], FP32, name="pl")
            for cit in range(4):
                nc.tensor.matmul(
                    pl[:],
                    lhsT=x4[:, cit, :],
                    rhs=wl_t[:, cit, :],
                    start=(cit == 0),
                    stop=(cit == 3),
                )
            out_sb = misc.tile([BL, 10], FP32, name="out_sb")
            nc.vector.tensor_add(out_sb[:], pl[:], bl_t[:])
            if accum_out:
                # accumulate into DRAM so R loop iterations yield R*result -
                # verifies the in-NEFF timing loop actually executed R times
                nc.gpsimd.dma_start(out_d[:], out_sb[:],
                                    accum_op=mybir.AluOpType.add)
            else:
                nc.sync.dma_start(out_d[:], out_sb[:])


def _prep_core_inputs(A, S, W1, b1, W2, b2, W3, b3, W4, b4, Wl, bl):
    """Host-side reshapes (pure data movement). Returns list of per-core dicts."""
    B = A.shape[0]
    # stage0 block-diagonal stationary from A:
    # abd[b, hw, g*4+c, n*16+g] = A[b, n*64+g*4+c, h, w]
    A_r = np.ascontiguousarray(
        A.reshape(B, 8, 16, 4, 4).transpose(0, 4, 2, 3, 1)
    )  # [B, hw, g, c, n]
    abd = np.zeros((B, 4, 64, P), dtype=ml_dtypes.bfloat16)
    abd_v = abd.reshape(B, 4, 16, 4, 8, 16)  # [B, hw, g, c, n, g']
    gg = np.arange(16)
    abd_v[:, :, gg, :, :, gg] = A_r.transpose(2, 0, 1, 3, 4)[gg].astype(
        ml_dtypes.bfloat16
    )
    # group-major staging layout: [B/4, part, b_in_group, hw*128]
    abd = np.ascontiguousarray(
        abd.reshape(B // 4, 4, 4, 64, P).transpose(0, 3, 1, 2, 4)
    ).reshape(B // 4, 64, 4, 4 * P)
    s2 = S.reshape(B, 64, 75).astype(ml_dtypes.bfloat16)
    s2 = np.ascontiguousarray(
        s2.reshape(B // 4, 4, 64, 75).transpose(0, 2, 1, 3)
    )

    def conv_w_tiles(W, n_cot, n_cit, k):
        co, ci = W.shape[0], W.shape[1]
        # -> [cot, cip, cit, kh, kw, cof]
        t = W.reshape(n_cot, P, n_cit, P, k, k).transpose(0, 3, 2, 4, 5, 1)
        return np.ascontiguousarray(t).astype(np.float32)

    # x0's channel layout from stage0 is (o, n*16+g); permute W1's ci to match
    # (reference ci index = n*48 + g*3 + o).
    o_i, n_i, g_i = np.meshgrid(
        np.arange(3), np.arange(8), np.arange(16), indexing="ij"
    )
    perm = (n_i * 48 + g_i * 3 + o_i).reshape(-1)
    w1t = conv_w_tiles(W1[:, perm], 3, 3, 3)
    w2t = conv_w_tiles(W2, 3, 3, 3)
    w3t = conv_w_tiles(W3, 4, 3, 3)
    w4t = conv_w_tiles(W4, 4, 4, 5).astype(ml_dtypes.bfloat16)

    wl_a = np.ascontiguousarray(
        Wl.T.reshape(4, P, 10).transpose(1, 0, 2)
    ).astype(np.float32)
    biases = np.zeros((P, 14), np.float32)
    biases[:, 0:3] = b1.reshape(3, P).T
    biases[:, 3:6] = b2.reshape(3, P).T
    biases[:, 6:10] = b3.reshape(4, P).T
    biases[:, 10:14] = b4.reshape(4, P).T
    blrep = np.tile(bl.astype(np.float32), (BL, 1))

    in_maps = []
    for c in range(NCORES):
        sl = slice(c * BL // 4, (c + 1) * BL // 4)
        in_maps.append({
            "abd": abd[sl], "s2": s2[sl],
            "w1t": w1t, "w2t": w2t, "w3t": w3t, "w4t": w4t,
            "wl": wl_a, "biases": biases, "blrep": blrep,
        })
    return in_maps


_PROGRAM_CACHE = {}


def _get_program():
    if "nc" not in _PROGRAM_CACHE:
        _PROGRAM_CACHE["nc"] = _build_program()
    return _PROGRAM_CACHE["nc"]


def kernel(A, S, W1, b1, W2, b2, W3, b3, W4, b4, Wl, bl):
    A = np.asarray(A, np.float32)
    S = np.asarray(S, np.float32)
    in_maps = _prep_core_inputs(
        A, S,
        np.asarray(W1, np.float32), np.asarray(b1, np.float32),
        np.asarray(W2, np.float32), np.asarray(b2, np.float32),
        np.asarray(W3, np.float32), np.asarray(b3, np.float32),
        np.asarray(W4, np.float32), np.asarray(b4, np.float32),
        np.asarray(Wl, np.float32), np.asarray(bl, np.float32),
    )
    nc = _get_program()
    res = run_bass_kernel_spmd(nc, in_maps, list(range(NCORES)))
    return np.concatenate([res.results[c]["OUT"] for c in range(NCORES)], axis=0)

